# revision 18
# baseline (speedup 1.0000x reference)
"""Trainium2 Bass kernel for GPLinear — Pauli rep + Karatsuba complex mult.

v3: on top of the M2(C) (Pauli) factorization (see v2), use the 3-mult
complex product:  m1 = aRe*bRe, m2 = aIm*bIm, m3 = (aRe+aIm)(bRe+bIm);
O_re = m1 - m2, O_im = m3 - m1 - m2.  The device accumulates the twelve
all-positive partial blocks M[v,(r,c)] = sum_{s,p} a(r,s,v) * w(s,c,v)
(v in {Re, Im, Sum}) — 48 matmuls of N=512, 25% fewer PE cycles than the
4-mult realification, and no sign handling on device at all.  The
m-recombination, inverse blade transform, and bias all happen on host.

Device layout (per core, bf16 operands, f32 PSUM):
  - xh: 8 coords uploaded (Re/Im per (r,s)); DVE forms the 4 Sum coords.
  - wh: 8 coords uploaded (Re/Im per (s,c)); DVE forms the 4 Sum coords.
  - PSUM: [b, (v, r, c, o)] = 3072 f32 = 6 banks.
  - DMA rings: xh on SP HWDGE, wh on ACT HWDGE, stores on GPSIMD SWDGE.

Sharding (8 cores): 4-way batch x 2-way out_features.
"""

import numpy as np
import ml_dtypes

import concourse.bass as bass
import concourse.mybir as mybir
import concourse.tile as tile
from concourse import bacc
from concourse.bass_utils import run_bass_kernel_spmd

F32 = mybir.dt.float32
BF16 = mybir.dt.bfloat16
BF16_NP = ml_dtypes.bfloat16

BATCH, IN_F, OUT_F, K8 = 512, 512, 512, 8
R_B, R_O = 4, 2
N_CORES = R_B * R_O
B_LOC = BATCH // R_B
OC = OUT_F // R_O
PT = IN_F // 128

LAST_RESULTS = None


def _cayley_table() -> np.ndarray:
    G = np.zeros((8, 8, 8), dtype=np.float32)
    for a in range(8):
        for b in range(8):
            swaps, t = 0, a >> 1
            while t:
                swaps += bin(t & b).count("1")
                t >>= 1
            G[a, b, a ^ b] = -1.0 if (swaps & 1) else 1.0
    return G


def _check_G(G):
    assert np.array_equal(np.asarray(G, dtype=np.float32), _cayley_table()), \
        "G is not the Cl(3,0) Cayley table this kernel hardcodes"


def _T8():
    T = np.zeros((8, 8), dtype=np.float32)
    rows = {
        (0, 0, 0): [(0, 1), (4, 1)],
        (0, 0, 1): [(3, 1), (7, 1)],
        (0, 1, 0): [(1, 1), (5, -1)],
        (0, 1, 1): [(6, 1), (2, -1)],
        (1, 0, 0): [(1, 1), (5, 1)],
        (1, 0, 1): [(2, 1), (6, 1)],
        (1, 1, 0): [(0, 1), (4, -1)],
        (1, 1, 1): [(7, 1), (3, -1)],
    }
    for (r, s, u), terms in rows.items():
        for i, coef in terms:
            T[r * 4 + s * 2 + u, i] = coef
    return T


def _S8():
    S = np.zeros((8, 8), dtype=np.float32)
    outrows = {
        0: [((0, 0, 0), 1), ((1, 1, 0), 1)],
        4: [((0, 0, 0), 1), ((1, 1, 0), -1)],
        3: [((0, 0, 1), 1), ((1, 1, 1), -1)],
        7: [((0, 0, 1), 1), ((1, 1, 1), 1)],
        1: [((0, 1, 0), 1), ((1, 0, 0), 1)],
        5: [((1, 0, 0), 1), ((0, 1, 0), -1)],
        2: [((1, 0, 1), 1), ((0, 1, 1), -1)],
        6: [((0, 1, 1), 1), ((1, 0, 1), 1)],
    }
    for k, terms in outrows.items():
        for (r, c, u), coef in terms:
            S[k, r * 4 + c * 2 + u] = coef
    return S


def build_kernel(G, loop_n=None, variant="full"):
    _check_G(G)
    nc = bacc.Bacc("TRN2", target_bir_lowering=False, debug=False)

    xh_d = nc.dram_tensor("xh", [128, PT * 8 * B_LOC], BF16,
                          kind="ExternalInput")
    wh_d = nc.dram_tensor("wh", [128, PT * 8 * OC], BF16,
                          kind="ExternalInput")
    o_d = nc.dram_tensor("out", [B_LOC, 12 * OC], BF16, kind="ExternalOutput")

    XD = 8 * B_LOC    # xh DMA cols per p-tile (Re, Im per (r,s))
    XS = 12 * B_LOC   # xh SBUF cols per p-tile (Re, Im, Sum per (r,s))
    WD = 8 * OC       # wh DMA cols per p-tile
    WS = 12 * OC      # wh SBUF cols per p-tile

    import contextlib

    NB = 2  # SBUF double-buffer sets; og (6 PSUM banks) is shared

    with tile.TileContext(nc) as tc:
        with (
            tc.tile_pool(name="sb", bufs=1) as sb,
            tc.tile_pool(name="ps", bufs=1, space="PSUM") as ps,
        ):
            # per (t, g): [Re | Im | Sum] blocks; g = (r,s) for xh, (s,c)
            # for wh.
            xh_t = [[sb.tile([128, XS], BF16, tag=f"xh{j}_{t}",
                             name=f"xh{j}_{t}") for t in range(PT)]
                    for j in range(NB)]
            wh_t = [[sb.tile([128, WS], BF16, tag=f"wh{j}_{t}",
                             name=f"wh{j}_{t}") for t in range(PT)]
                    for j in range(NB)]
            out_sb = [sb.tile([128, 12 * OC], BF16, tag=f"out{j}",
                              name=f"out{j}") for j in range(NB)]
            og = ps.tile([128, 12 * OC], F32, tag="og")  # (v,r,c,o) 6 banks

            def block3(tile_t, width, blk):
                pitch = tile_t[:].ap[0][0]
                return bass.AP(tensor=tile_t.tensor, offset=blk * width,
                               ap=[[pitch, 128], [3 * width, 4], [1, width]])

            def do_dma(j, sums=True):
                for t in range(PT):
                    x_, w_ = xh_t[j][t], wh_t[j][t]
                    pitch_xh = x_[:].ap[0][0]
                    pitch_wh = w_[:].ap[0][0]
                    dstx = bass.AP(tensor=x_.tensor, offset=0,
                                   ap=[[pitch_xh, 128], [3 * B_LOC, 4],
                                       [1, 2 * B_LOC]])
                    nc.sync.dma_start(dstx,
                                      xh_d.ap()[:, t * XD:(t + 1) * XD])
                    dstw = bass.AP(tensor=w_.tensor, offset=0,
                                   ap=[[pitch_wh, 128], [3 * OC, 4],
                                       [1, 2 * OC]])
                    nc.sync.dma_start(dstw,
                                      wh_d.ap()[:, t * WD:(t + 1) * WD])
                    if not sums:
                        continue
                    # DVE: Sum = Re + Im
                    nc.vector.tensor_tensor(
                        out=block3(x_, B_LOC, 2), in0=block3(x_, B_LOC, 0),
                        in1=block3(x_, B_LOC, 1), op=mybir.AluOpType.add)
                    nc.vector.tensor_tensor(
                        out=block3(w_, OC, 2), in0=block3(w_, OC, 0),
                        in1=block3(w_, OC, 1), op=mybir.AluOpType.add)

            # ---- 48 matmuls, all N=512: M[v,(r,c)] += a(r,s,v)^T w(s,c,v)
            def do_mms(j):
                for t in range(PT):
                    x_, w_ = xh_t[j][t], wh_t[j][t]
                    pitch_xh = x_[:].ap[0][0]
                    pitch_wh = w_[:].ap[0][0]
                    for s in range(2):
                        for r in range(2):
                            first = (t == 0 and s == 0)
                            last = (t == PT - 1 and s == 1)
                            for v in range(3):
                                a_col = ((r * 2 + s) * 3 + v) * B_LOC
                                rhs = bass.AP(
                                    tensor=w_.tensor,
                                    offset=s * 6 * OC + v * OC,
                                    ap=[[pitch_wh, 128], [3 * OC, 2],
                                        [1, OC]])
                                ooff = v * 1024 + r * 512
                                nc.tensor.matmul(
                                    og[:, ooff:ooff + 512],
                                    bass.AP(tensor=x_.tensor, offset=a_col,
                                            ap=[[pitch_xh, 128],
                                                [1, B_LOC]]),
                                    rhs,
                                    start=first, stop=last)

            # ---- evacuation: PSUM -> SBUF bf16, ACT early half + DVE late
            def do_evac(j, store=True):
                nc.scalar.copy(out_sb[j][:, 0:1536], og[:, 0:1536])
                if store:
                    nc.gpsimd.dma_start(o_d.ap()[:, 0:1536],
                                        out_sb[j][:, 0:1536])
                nc.vector.tensor_copy(out_sb[j][:, 1536:3072],
                                      og[:, 1536:3072])
                if store:
                    nc.gpsimd.dma_start(o_d.ap()[:, 1536:3072],
                                        out_sb[j][:, 1536:3072])

            def body(j, store=True):
                do_dma(j)
                do_mms(j)
                do_evac(j, store=store)

            if loop_n:
                assert loop_n % NB == 0, f"loop_n must be a multiple of {NB}"
            loop = (tc.For_i(0, loop_n // NB, 1) if loop_n
                    else contextlib.nullcontext())
            if variant == "full":
                if not loop_n:
                    body(0)
                else:
                    with loop:
                        for j in range(NB):
                            body(j)
            elif variant == "mm":
                for j in range(NB):
                    do_dma(j)
                with loop:
                    for j in range(NB):
                        do_mms(j)
                        do_evac(j, store=False)
            elif variant == "dma":
                with loop:
                    for j in range(NB):
                        do_dma(j, sums=False)
            else:
                raise ValueError(variant)

    nc.compile()
    return nc


def _host_transform(x, W, b=None):
    x = np.asarray(x, dtype=np.float32)
    W = np.asarray(W, dtype=np.float32)
    T8 = _T8()

    xh8 = np.einsum("bpi,ai->bpa", x, T8).astype(BF16_NP)   # [B,P,8] (r,s,u)
    wh8 = np.einsum("poj,aj->poa", W, 0.5 * T8).astype(BF16_NP)  # (s,c,u')

    in_maps = []
    for c in range(N_CORES):
        bc, oc = divmod(c, R_O)
        xh_c = xh8[bc * B_LOC:(bc + 1) * B_LOC]           # [128, 512, 8]
        xh_c = np.ascontiguousarray(
            xh_c.transpose(1, 2, 0)                        # [512, 8, 128]
                .reshape(PT, 128, 8, B_LOC)                # [t, p, (r,s,u), b]
                .transpose(1, 0, 2, 3)
                .reshape(128, PT * 8 * B_LOC))
        wh_c = wh8[:, oc * OC:(oc + 1) * OC, :]           # [512, 256, 8]
        wh_c = np.ascontiguousarray(
            wh_c.transpose(0, 2, 1)                        # [512, 8, 256]
                .reshape(PT, 128, 8, OC)                   # [t, p, (s,c,u'), o]
                .transpose(1, 0, 2, 3)
                .reshape(128, PT * 8 * OC))
        in_maps.append({"xh": xh_c, "wh": wh_c})
    return in_maps


def make_in_maps(x, W, b, G=None):
    return _host_transform(x, W, b)


_CACHE = {}


def kernel(x, W, b, G):
    global LAST_RESULTS
    _check_G(G)
    if "nc" not in _CACHE:
        _CACHE["nc"] = build_kernel(G)
    nc = _CACHE["nc"]

    in_maps = _host_transform(x, W)
    res = run_bass_kernel_spmd(nc, in_maps, core_ids=list(range(N_CORES)))
    LAST_RESULTS = res

    S8 = _S8()
    b = np.asarray(b, dtype=np.float32)
    out = np.empty((BATCH, OUT_F, K8), dtype=np.float32)
    for c in range(N_CORES):
        bc, oc = divmod(c, R_O)
        M = np.asarray(res.results[c]["out"]).astype(np.float32)
        M = M.reshape(B_LOC, 3, 2, 2, OC)                 # [b, v, r, c, o]
        O = np.empty((B_LOC, 2, 2, 2, OC), dtype=np.float32)  # [b,r,c,u'',o]
        O[:, :, :, 0] = M[:, 0] - M[:, 1]                 # m1 - m2
        O[:, :, :, 1] = M[:, 2] - M[:, 0] - M[:, 1]       # m3 - m1 - m2
        O = O.reshape(B_LOC, 8, OC)                       # gamma = (r,c,u'')
        o_c = np.einsum("kg,bgo->bok", S8, O) + b[oc * OC:(oc + 1) * OC]
        out[bc * B_LOC:(bc + 1) * B_LOC, oc * OC:(oc + 1) * OC, :] = o_c
    return out
